# revision 45
# baseline (speedup 1.0000x reference)
"""Multi-head self-attention on 8 Trainium2 NeuronCores.

Problem: x[2, 2048, 1024], 16 heads, Dh=64, fp32.
  q/k/v = x @ W.T ; scores = q k^T / 8 ; out = softmax(scores) v @ W_o.T

Sharding (Megatron-style): each core owns 2 heads (128 of the 1024 model
dims). W_q/W_k/W_v column-sharded, W_o row-sharded; the cross-core
all-reduce of the output-projection partials is done on the host.

Per-core layout strategy (v2, bf16 datapath):
  - x is fed pre-transposed AND pre-cast to bf16 (xT [1024, 4096]), halving
    input DMA. Loaded as 16 resident [128, 2048] SBUF slices (2 queues).
  - qT/kTt computed as [128 = 2 heads x 64, 4096] bf16 (dims on partitions).
  - Scores computed transposed, S^T[k_tok, q_tok], heads packed in PE
    row-groups (K=64 each at partition bases 0/64) -> one [128, 1024] f32
    2-bank PSUM tile per k-strip; exp() on ScalarE reads it, writes bf16.
  - v in natural layout via PE transpose (bf16, 1 cycle/row), augmented
    with a ones column: PV matmul (M=65) yields the weighted values and the
    softmax denominators in one accumulation chain.
  - Softmax normalization fully on-chip (no DRAM round trips): VectorE
    reciprocal reads the denominator rows straight from PSUM into partition
    rows 64/65 of an SBUF tile; one K=2 PE matmul against a {0,1} selector
    broadcasts [rA; rB] to a [128, 512] PSUM tile; one tensor_mul
    normalizes. Output projection runs on the normalized f32r tile.
  - A short chain of warm-up matmuls at t=0 ramps the PE p-state to 2.4GHz
    before the first real chain (cold-start matmuls otherwise run 2-4x
    slower and every stall re-ramps).
  - Filler work (next batch's QKV, transposes, output projections) is
    interleaved into the exp-paced strip loop with a cost-credit scheduler
    so the PE never idles while ScalarE runs exp.
  - Output partials are written bf16 (halved output DMA); host reduces in
    fp32.
"""

import os
from collections import deque
from contextlib import ExitStack

import numpy as np

import concourse.bass as bass
import concourse.tile as tile
from concourse import bacc, mybir
from concourse._compat import with_exitstack
from concourse.bass_utils import run_bass_kernel_spmd

F32 = mybir.dt.float32
F32R = mybir.dt.float32r
BF16 = mybir.dt.bfloat16
EXP = mybir.ActivationFunctionType.Exp

P = 128          # partitions / head-pair dims per core
D = 1024         # model dim
T = 2048         # tokens per batch
NB = 2           # batches
BT = NB * T      # 4096 flattened tokens
KT = D // P      # 8 contraction tiles over model dim
NQ = T // 512    # 4 q-tiles of 512 per batch
NS = T // P      # 16 k-strips of 128 per batch
N_CORES = 8
N_WARM = 16      # p-state warm-up matmuls


@with_exitstack
def _mhsa_kernel(ctx: ExitStack, tc: tile.TileContext, out, xT, wq_in, wk_in,
                 wv_in, wo_in, ident_in, ones_in):
    nc = tc.nc

    # ---- pools ----
    wpool = ctx.enter_context(tc.tile_pool(name="persist", bufs=1))
    expool = ctx.enter_context(tc.tile_pool(name="expp", bufs=4))
    # one buffer per q-tile (8 total): never ring-recycled, so no slot wait
    # can point forward in PE program order (deadlock-free by construction)
    rrpool = ctx.enter_context(tc.tile_pool(name="rrp", bufs=NB * NQ))
    aupool = ctx.enter_context(tc.tile_pool(name="aup", bufs=NB * NQ))
    anpool = ctx.enter_context(tc.tile_pool(name="anp", bufs=NB * NQ))
    bcpool = ctx.enter_context(tc.tile_pool(name="bcp", bufs=NB * NQ))
    dspool = ctx.enter_context(tc.tile_pool(name="dsp", bufs=2))
    rrpool2 = ctx.enter_context(tc.tile_pool(name="rrp2", bufs=2))
    drpool = ctx.enter_context(tc.tile_pool(name="drp", bufs=NB * NQ,
                                            space="DRAM"))
    otpool = ctx.enter_context(tc.tile_pool(name="otp", bufs=4))

    ps_sc = ctx.enter_context(tc.tile_pool(name="ps_sc", bufs=2, space="PSUM"))
    ps_av = ctx.enter_context(tc.tile_pool(name="ps_av", bufs=2, space="PSUM"))
    ps_m = ctx.enter_context(tc.tile_pool(name="ps_m", bufs=2, space="PSUM"))

    # ---- persistent tiles (name-keyed slots in wpool) ----
    warm_sb = wpool.tile([P, 512], BF16, name="warm_sb")
    ident = wpool.tile([P, P], BF16, name="ident")
    wq_sb = wpool.tile([P, D], BF16, name="wq_sb")
    wk_sb = wpool.tile([P, D], BF16, name="wk_sb")
    wv_sb = wpool.tile([P, D], BF16, name="wv_sb")
    wo_sb = wpool.tile([P, D], F32R, name="wo_sb")
    qT = wpool.tile([P, BT], BF16, name="qT")
    kTt = wpool.tile([P, BT], BF16, name="kTt")
    vts = [wpool.tile([P, T], BF16, name=f"vt{b}") for b in range(NB)]
    vas = {(b, h): wpool.tile([P, NS * 65], BF16, name=f"va{h}{b}")
           for b in range(NB) for h in ("A", "B")}
    xcs = {(b, k): wpool.tile([P, T], BF16, name=f"xc{b}_{k}")
           for b in range(NB) for k in range(KT)}

    # ---- t=0: memsets, DMA issues, PE warm-up ----
    nc.vector.memset(warm_sb[:], 0.0)
    for b in range(NB):
        for h in ("A", "B"):
            va = vas[(b, h)]
            dst = va.rearrange("p (s c) -> p s c", c=65)[:, :, 64:65]
            src = ones_in.rearrange("p (s c) -> p s c", c=1)
            nc.gpsimd.dma_start(out=dst, in_=src)

    # scalar queue: consts + weights
    nc.scalar.dma_start(out=ident[:], in_=ident_in[:])
    nc.scalar.dma_start(out=wq_sb[:], in_=wq_in[:])
    nc.scalar.dma_start(out=wk_sb[:], in_=wk_in[:])
    nc.scalar.dma_start(out=wv_sb[:], in_=wv_in[:])
    nc.scalar.dma_start(out=wo_sb[:], in_=wo_in[:])
    # x slices in [128, 1024] chunks striped over three DMA queues, in the
    # order phase 1 consumes them (b0 cols 0:1024 first, then 1024:2048, ...)
    dqs = [nc.sync, nc.scalar, nc.gpsimd]
    qi = 0
    for b in range(NB):
        for half in range(2):
            lo = b * T + half * 1024
            for k in range(KT):
                dqs[qi % 3].dma_start(
                    out=xcs[(b, k)][:, half * 1024:half * 1024 + 1024],
                    in_=xT[k * P:(k + 1) * P, lo:lo + 1024])
                qi += 1

    # warm-up: keep PE busy (and ramping) while x streams in
    for i in range(N_WARM):
        wps = ps_m.tile([P, 512], F32, name="warm_ps", tag="m")
        nc.tensor.matmul(wps[:], warm_sb[:, 0:P], warm_sb[:], start=True,
                         stop=True)

    # ---- QKV projection chains ----
    def chain_parts(b, n, which):
        """One [128, 512] projection tile as two 4-matmul filler chunks."""
        cell = {}
        w_sb = {"q": wq_sb, "k": wk_sb, "v": wv_sb}[which]

        def f1():
            acc = ps_m.tile([P, 512], F32, name="qkv_ps", tag="m")
            cell["acc"] = acc
            for k in range(KT // 2):
                nc.tensor.matmul(
                    acc[:], w_sb[:, k * P:(k + 1) * P],
                    xcs[(b, k)][:, n * 512:(n + 1) * 512],
                    start=(k == 0), stop=False,
                )

        def f2():
            acc = cell["acc"]
            for k in range(KT // 2, KT):
                nc.tensor.matmul(
                    acc[:], w_sb[:, k * P:(k + 1) * P],
                    xcs[(b, k)][:, n * 512:(n + 1) * 512],
                    start=False, stop=(k == KT - 1),
                )
            col = b * T + n * 512
            dst, dcol = {
                "q": (qT, col), "k": (kTt, col), "v": (vts[b], n * 512),
            }[which]
            nc.vector.tensor_copy(dst[:, dcol:dcol + 512], acc[:])
        # 'f1' kind: the PSUM accumulator stays open until f2 runs, so the
        # pop loop must emit f2 before any other ps_m user.
        return [(f1, 852, "f1"), (f2, 852, "f2")]

    def trans(b, s):
        """PE-transpose one 128-token strip of v into the augmented layout."""
        def f():
            tp = ps_m.tile([P, P], BF16, name="tr_ps", tag="m")
            nc.tensor.transpose(tp[:], vts[b][:, s * P:(s + 1) * P], ident[:])
            nc.vector.tensor_copy(vas[(b, "A")][:, s * 65:s * 65 + 64],
                                  tp[:, 0:64])
            nc.vector.tensor_copy(vas[(b, "B")][:, s * 65:s * 65 + 64],
                                  tp[:, 64:128])
        return [(f, 150, "x")]

    def qkv_chunks(b, parts=("q", "k", "v"), ns=range(NQ), with_trans=True):
        chunks = []
        if "k" in parts:
            for n in ns:
                chunks.extend(chain_parts(b, n, "k"))
        if "v" in parts:
            for n in ns:
                chunks.extend(chain_parts(b, n, "v"))
                if with_trans:
                    for s in range(4 * n, 4 * n + 4):
                        chunks.extend(trans(b, s))
        if "q" in parts:
            for n in ns:
                chunks.extend(chain_parts(b, n, "q"))
        return chunks

    # ---- per-q-tile state for drain/norm/outproj ----
    tile_state = {}

    def drain_norm(b, n, queue):
        """Right after the PV stop, before the av PSUM slots are re-claimed:
        fast reciprocals of the denominator rows (VectorE, direct from PSUM),
        DMA the unnormalized values out, broadcast the reciprocal rows via
        SBUF-to-SBUF DMA, one multiply to normalize. No PE instructions, so
        nothing here can stall the strip pipeline."""
        st = tile_state[(b, n)]
        avA, avB = st["avA"], st["avB"]
        dstg = dspool.tile([P, 1024], F32, name="dstg", tag="ds")
        nc.vector.tensor_copy(dstg[0:1, 0:512], avA[64:65, :])
        nc.vector.tensor_copy(dstg[0:1, 512:1024], avB[64:65, :])
        av_un = aupool.tile([P, 512], F32, name="av_un", tag="aun")
        nc.vector.tensor_copy(av_un[0:64, :], avA[0:64, :])
        nc.vector.tensor_copy(av_un[64:128, :], avB[0:64, :])
        # custom-DVE op only works at partition base 0
        rr = rrpool2.tile([P, 1024], F32, name="rr", tag="rr")
        nc.vector.reciprocal_approx_fast(out=rr[0:1, :], in_=dstg[0:1, :])
        rscr = drpool.tile([2, 512], F32, name="rscr", tag="rscr")
        nc.sync.dma_start(out=rscr[0:1, :], in_=rr[0:1, 0:512])
        nc.gpsimd.dma_start(out=rscr[1:2, :], in_=rr[0:1, 512:1024])
        bc = bcpool.tile([P, 512], F32, name="bc_sb", tag="bc")
        nc.sync.dma_start(out=bc[0:64, :],
                          in_=rscr[0:1, :].to_broadcast((64, 512)))
        nc.gpsimd.dma_start(out=bc[64:128, :],
                            in_=rscr[1:2, :].to_broadcast((64, 512)))
        av_n = anpool.tile([P, 512], F32R, name="av_n", tag="an")
        nc.vector.tensor_mul(av_n[:], av_un[:], bc[:])
        st["av_n"] = av_n
        for sub in range(4):
            queue.extend(outproj(b, n, sub))

    def outproj(b, n, sub):
        def f():
            av_n = tile_state[(b, n)]["av_n"]
            ot = otpool.tile([P, D], BF16, name="ot", tag="ot")
            for jh in range(2):
                op = ps_m.tile([P, 512], F32, name="op_ps", tag="m")
                nc.tensor.matmul(
                    op[:], av_n[:, sub * P:(sub + 1) * P],
                    wo_sb[:, jh * 512:(jh + 1) * 512],
                    start=True, stop=True,
                )
                nc.vector.tensor_copy(ot[:, jh * 512:(jh + 1) * 512], op[:])
            row0 = b * T + n * 512 + sub * P
            eng = nc.sync if sub % 2 == 0 else nc.gpsimd
            eng.dma_start(out=out[row0:row0 + P, :], in_=ot[:])
        return [(f, 500, "x")]

    # ---- exp-paced attention strip loop with credit-based filler ----
    CREDIT_PER_STRIP = 310
    CREDIT_CAP = 1000
    sched = {"credit": 0, "forced": None, "pv": None}

    def pump(queue, add):
        credit = min(sched["credit"] + add, CREDIT_CAP)
        if sched["forced"] is not None:
            f, cost, _ = sched["forced"]
            f()
            credit -= cost
            sched["forced"] = None
        while queue and queue[0][1] <= credit:
            f, cost, kind = queue.popleft()
            f()
            credit -= cost
            if kind == "f1":
                # matching f2 must be the next ps_m user
                sched["forced"] = queue.popleft()
                break
        sched["credit"] = credit

    def attn_batch(b, queue, on_tile_start=None):
        for n in range(NQ):
            if on_tile_start is not None:
                on_tile_start(n, queue)
            qcol = b * T + n * 512
            avA = ps_av.tile([P, 512], F32, name="avA", tag="av")
            avB = ps_av.tile([P, 512], F32, name="avB", tag="av")
            tile_state[(b, n)] = {"avA": avA, "avB": avB}
            for s in range(NS):
                kcol = b * T + s * P
                sc = ps_sc.tile([P, 1024], F32, name="sc", tag="sc")
                nc.tensor.matmul(
                    sc[:, 0:512], kTt[0:64, kcol:kcol + P],
                    qT[0:64, qcol:qcol + 512], start=True, stop=True,
                )
                nc.tensor.matmul(
                    sc[:, 512:1024], kTt[64:128, kcol:kcol + P],
                    qT[64:128, qcol:qcol + 512], start=True, stop=True,
                )
                ex = expool.tile([P, 1024], BF16, name="ex", tag="ex")
                nc.scalar.activation(out=ex[:], in_=sc[:], func=EXP,
                                     scale=0.125)
                # PE filler, then the PREVIOUS strip's PV matmuls: by the
                # time the PE reaches them, exp(s-1) is long done, so the
                # PE never waits on ScalarE and stays at max p-state.
                pump(queue, CREDIT_PER_STRIP)
                if sched["pv"] is not None:
                    sched["pv"]()

                def pv(s=s, ex=ex, avA=avA, avB=avB, b=b):
                    nc.tensor.matmul(
                        avA[0:65, :], vas[(b, "A")][:, s * 65:(s + 1) * 65],
                        ex[:, 0:512], start=(s == 0), stop=(s == NS - 1),
                    )
                    nc.tensor.matmul(
                        avB[0:65, :], vas[(b, "B")][:, s * 65:(s + 1) * 65],
                        ex[:, 512:1024], start=(s == 0), stop=(s == NS - 1),
                    )
                sched["pv"] = pv
            # boundary: one extra filler beat so the final PV's exp is done
            pump(queue, 500)
            sched["pv"]()
            sched["pv"] = None
            drain_norm(b, n, queue)
        return queue

    # ---- schedule ----
    # Phase 1: batch-0 QKV (PE-serial, overlapped with x DMA)
    for f, _c, _k in qkv_chunks(0):
        f()

    # Phase 2: attention batch 0; filler = batch-1 k/v chains + transposes
    # + q(b1, n0), then batch-0 outprojs as tiles complete.
    q2 = deque(qkv_chunks(1, parts=("k", "v")) +
               qkv_chunks(1, parts=("q",), ns=[0]))
    leftover = attn_batch(0, q2)

    # Phase 3: attention batch 1; filler = leftovers + q(b1, n+1) chains
    # (prepended at tile starts) + outprojs.
    def tile_start(n, queue):
        if n + 1 < NQ:
            for item in reversed(qkv_chunks(1, parts=("q",), ns=[n + 1])):
                queue.appendleft(item)

    q3 = leftover
    attn_batch(1, q3, on_tile_start=tile_start)

    # Tail: flush remaining filler (ends with the last tile's outproj).
    if sched["forced"] is not None:
        sched["forced"][0]()
        sched["forced"] = None
    while q3:
        f, _, kind = q3.popleft()
        f()


_PROGRAM = None


def _build_program():
    nc = bacc.Bacc(
        "TRN2", target_bir_lowering=False, debug=False,
        enable_asserts=False, num_devices=N_CORES,
    )
    xT = nc.dram_tensor("xT", [D, BT], BF16, kind="ExternalInput").ap()
    wq_in = nc.dram_tensor("wq_in", [P, D], BF16, kind="ExternalInput").ap()
    wk_in = nc.dram_tensor("wk_in", [P, D], BF16, kind="ExternalInput").ap()
    wv_in = nc.dram_tensor("wv_in", [P, D], BF16, kind="ExternalInput").ap()
    wo_in = nc.dram_tensor("wo_in", [P, D], F32R, kind="ExternalInput").ap()
    ident_in = nc.dram_tensor("ident_in", [P, P], BF16,
                              kind="ExternalInput").ap()
    ones_in = nc.dram_tensor("ones_in", [P, NS], BF16,
                             kind="ExternalInput").ap()
    out = nc.dram_tensor("out", [BT, D], BF16, kind="ExternalOutput").ap()
    with tile.TileContext(nc) as tc:
        _mhsa_kernel(tc, out, xT, wq_in, wk_in, wv_in, wo_in, ident_in,
                     ones_in)
    nc.compile()
    return nc


def get_program():
    global _PROGRAM
    if _PROGRAM is None:
        _PROGRAM = _build_program()
    return _PROGRAM


last_results = None


def _install_trace_hook():
    """Register the axon NTFF-profile hook that the agent image's antenv
    lacks, so run_bass_kernel_spmd(trace=True) can capture HW timings."""
    import sys
    import types

    if "antenv.axon_hooks" in sys.modules:
        return
    try:
        from trn_agent_boot.trn_boot import _ntff_profile_via_ctypes
        hook = _ntff_profile_via_ctypes("/opt/axon/libaxon_pjrt.so")
    except Exception:
        hook = None
    mod = types.ModuleType("antenv.axon_hooks")
    state = {"hook": hook}
    mod.get_axon_ntff_profile_hook = lambda: state["hook"]
    mod.set_axon_ntff_profile_hook = lambda h: state.__setitem__("hook", h)
    sys.modules["antenv.axon_hooks"] = mod

    import concourse.bass_utils as bu
    orig_upload = bu.upload_artifacts

    def safe_upload(tmpdir):
        try:
            return orig_upload(tmpdir)
        except Exception:
            return tmpdir

    bu.upload_artifacts = safe_upload


def kernel(x, W_q, W_k, W_v, W_o):
    global last_results
    import ml_dtypes
    bf16 = ml_dtypes.bfloat16

    x = np.ascontiguousarray(np.asarray(x, dtype=np.float32))
    W_q = np.asarray(W_q, dtype=np.float32)
    W_k = np.asarray(W_k, dtype=np.float32)
    W_v = np.asarray(W_v, dtype=np.float32)
    W_o = np.asarray(W_o, dtype=np.float32)

    xTn = np.ascontiguousarray(x.reshape(BT, D).T.astype(bf16))
    ident = np.eye(P, dtype=np.float32).astype(bf16)
    ones16 = np.ones((P, NS), dtype=np.float32).astype(bf16)

    def pack_w(w_rows):
        # [1024 in-dims, 128 out-dims] -> [128, 8*128] with the k-slice
        # index folded into the free dim (one contiguous DMA).
        wT = np.ascontiguousarray(w_rows.T)          # [1024, 128]
        return np.ascontiguousarray(
            wT.reshape(KT, P, P).transpose(1, 0, 2).reshape(P, D).astype(bf16)
        )

    in_maps = []
    for c in range(N_CORES):
        sl = slice(P * c, P * (c + 1))
        in_maps.append({
            "xT": xTn,
            "wq_in": pack_w(W_q[sl, :]),
            "wk_in": pack_w(W_k[sl, :]),
            "wv_in": pack_w(W_v[sl, :]),
            "wo_in": np.ascontiguousarray(W_o[:, sl].T),
            "ident_in": ident,
            "ones_in": ones16,
        })

    trace = bool(int(os.environ.get("KERNEL_TRACE", "0")))
    if trace:
        _install_trace_hook()
    nc = get_program()
    res = run_bass_kernel_spmd(
        nc, in_maps, core_ids=list(range(N_CORES)), trace=trace,
    )
    last_results = res
    total = res.results[0]["out"].astype(np.float32)
    for r in res.results[1:]:
        total = total + r["out"].astype(np.float32)
    return total.reshape(NB, T, D)


# revision 51
# speedup vs baseline: 1.0274x; 1.0274x over previous
"""Multi-head self-attention on 8 Trainium2 NeuronCores.

Problem: x[2, 2048, 1024], 16 heads, Dh=64, fp32.
  q/k/v = x @ W.T ; scores = q k^T / 8 ; out = softmax(scores) v @ W_o.T

Sharding (Megatron-style): each core owns 2 heads (128 of the 1024 model
dims). W_q/W_k/W_v column-sharded, W_o row-sharded; the cross-core
all-reduce of the output-projection partials is done on the host.

Per-core layout strategy (v2, bf16 datapath):
  - x is fed pre-transposed AND pre-cast to bf16 (xT [1024, 4096]), halving
    input DMA. Loaded as 16 resident [128, 2048] SBUF slices (2 queues).
  - qT/kTt computed as [128 = 2 heads x 64, 4096] bf16 (dims on partitions).
  - Scores computed transposed, S^T[k_tok, q_tok], heads packed in PE
    row-groups (K=64 each at partition bases 0/64) -> one [128, 1024] f32
    2-bank PSUM tile per k-strip; exp() on ScalarE reads it, writes bf16.
  - v in natural layout via PE transpose (bf16, 1 cycle/row), augmented
    with a ones column: PV matmul (M=65) yields the weighted values and the
    softmax denominators in one accumulation chain.
  - Softmax normalization fully on-chip (no DRAM round trips): VectorE
    reciprocal reads the denominator rows straight from PSUM into partition
    rows 64/65 of an SBUF tile; one K=2 PE matmul against a {0,1} selector
    broadcasts [rA; rB] to a [128, 512] PSUM tile; one tensor_mul
    normalizes. Output projection runs on the normalized f32r tile.
  - A short chain of warm-up matmuls at t=0 ramps the PE p-state to 2.4GHz
    before the first real chain (cold-start matmuls otherwise run 2-4x
    slower and every stall re-ramps).
  - Filler work (next batch's QKV, transposes, output projections) is
    interleaved into the exp-paced strip loop with a cost-credit scheduler
    so the PE never idles while ScalarE runs exp.
  - Output partials are written bf16 (halved output DMA); host reduces in
    fp32.
"""

import os
from collections import deque
from contextlib import ExitStack

import numpy as np

import concourse.bass as bass
import concourse.tile as tile
from concourse import bacc, mybir
from concourse._compat import with_exitstack
from concourse.bass_utils import run_bass_kernel_spmd

F32 = mybir.dt.float32
F32R = mybir.dt.float32r
BF16 = mybir.dt.bfloat16
EXP = mybir.ActivationFunctionType.Exp

P = 128          # partitions / head-pair dims per core
D = 1024         # model dim
T = 2048         # tokens per batch
NB = 2           # batches
BT = NB * T      # 4096 flattened tokens
KT = D // P      # 8 contraction tiles over model dim
NQ = T // 512    # 4 q-tiles of 512 per batch
NS = T // P      # 16 k-strips of 128 per batch
N_CORES = 8
N_WARM = 10      # p-state warm-up matmuls


@with_exitstack
def _mhsa_kernel(ctx: ExitStack, tc: tile.TileContext, out, xT, wq_in, wk_in,
                 wv_in, wo_in, ident_in, ones_in):
    nc = tc.nc

    # ---- pools ----
    wpool = ctx.enter_context(tc.tile_pool(name="persist", bufs=1))
    expool = ctx.enter_context(tc.tile_pool(name="expp", bufs=4))
    # one buffer per q-tile (8 total): never ring-recycled, so no slot wait
    # can point forward in PE program order (deadlock-free by construction)
    rrpool = ctx.enter_context(tc.tile_pool(name="rrp", bufs=NB * NQ))
    aupool = ctx.enter_context(tc.tile_pool(name="aup", bufs=NB * NQ))
    anpool = ctx.enter_context(tc.tile_pool(name="anp", bufs=NB * NQ))
    bcpool = ctx.enter_context(tc.tile_pool(name="bcp", bufs=NB * NQ))
    dspool = ctx.enter_context(tc.tile_pool(name="dsp", bufs=2))
    rrpool2 = ctx.enter_context(tc.tile_pool(name="rrp2", bufs=2))
    drpool = ctx.enter_context(tc.tile_pool(name="drp", bufs=NB * NQ,
                                            space="DRAM"))
    otpool = ctx.enter_context(tc.tile_pool(name="otp", bufs=4))

    ps_sc = ctx.enter_context(tc.tile_pool(name="ps_sc", bufs=2, space="PSUM"))
    ps_av = ctx.enter_context(tc.tile_pool(name="ps_av", bufs=2, space="PSUM"))
    ps_m = ctx.enter_context(tc.tile_pool(name="ps_m", bufs=2, space="PSUM"))

    # ---- persistent tiles (name-keyed slots in wpool) ----
    warm_sb = wpool.tile([P, 512], BF16, name="warm_sb")
    ident = wpool.tile([P, P], BF16, name="ident")
    wq_sb = wpool.tile([P, D], BF16, name="wq_sb")
    wk_sb = wpool.tile([P, D], BF16, name="wk_sb")
    wv_sb = wpool.tile([P, D], BF16, name="wv_sb")
    wo_sb = wpool.tile([P, D], F32R, name="wo_sb")
    qT = wpool.tile([P, BT], BF16, name="qT")
    kTt = wpool.tile([P, BT], BF16, name="kTt")
    vts = [wpool.tile([P, T], BF16, name=f"vt{b}") for b in range(NB)]
    vas = {(b, h): wpool.tile([P, NS * 65], BF16, name=f"va{h}{b}")
           for b in range(NB) for h in ("A", "B")}
    xcs = {(b, k): wpool.tile([P, T], BF16, name=f"xc{b}_{k}")
           for b in range(NB) for k in range(KT)}

    # ---- t=0: memsets, DMA issues, PE warm-up ----
    nc.vector.memset(warm_sb[:], 0.0)

    # weights needed first on scalar; x slices in [128, 1024] chunks striped
    # over three DMA queues in the order phase 1 consumes them; slow-issue
    # scatter DMAs (va ones columns) go last on gpsimd (needed ~35us in).
    nc.scalar.dma_start(out=ident[:], in_=ident_in[:])
    nc.scalar.dma_start(out=wq_sb[:], in_=wq_in[:])
    dqs = [nc.sync, nc.scalar, nc.gpsimd]
    qi = 0
    for b in range(NB):
        for half in range(2):
            lo = b * T + half * 1024
            for k in range(KT):
                dqs[qi % 3].dma_start(
                    out=xcs[(b, k)][:, half * 1024:half * 1024 + 1024],
                    in_=xT[k * P:(k + 1) * P, lo:lo + 1024])
                qi += 1
            if b == 0 and half == 0:
                nc.scalar.dma_start(out=wk_sb[:], in_=wk_in[:])
            elif b == 0 and half == 1:
                nc.scalar.dma_start(out=wv_sb[:], in_=wv_in[:])
                nc.scalar.dma_start(out=wo_sb[:], in_=wo_in[:])
    for b in range(NB):
        for h in ("A", "B"):
            va = vas[(b, h)]
            dst = va.rearrange("p (s c) -> p s c", c=65)[:, :, 64:65]
            src = ones_in.rearrange("p (s c) -> p s c", c=1)
            nc.gpsimd.dma_start(out=dst, in_=src)

    # warm-up: keep PE busy (and ramping) while x streams in
    for i in range(N_WARM):
        wps = ps_m.tile([P, 512], F32, name="warm_ps", tag="m")
        nc.tensor.matmul(wps[:], warm_sb[:, 0:P], warm_sb[:], start=True,
                         stop=True)

    # ---- QKV projection chains ----
    def chain_parts(b, n, which):
        """One [128, 512] projection tile as two 4-matmul filler chunks."""
        cell = {}
        w_sb = {"q": wq_sb, "k": wk_sb, "v": wv_sb}[which]

        def f1():
            acc = ps_m.tile([P, 512], F32, name="qkv_ps", tag="m")
            cell["acc"] = acc
            for k in range(KT // 2):
                nc.tensor.matmul(
                    acc[:], w_sb[:, k * P:(k + 1) * P],
                    xcs[(b, k)][:, n * 512:(n + 1) * 512],
                    start=(k == 0), stop=False,
                )

        def f2():
            acc = cell["acc"]
            for k in range(KT // 2, KT):
                nc.tensor.matmul(
                    acc[:], w_sb[:, k * P:(k + 1) * P],
                    xcs[(b, k)][:, n * 512:(n + 1) * 512],
                    start=False, stop=(k == KT - 1),
                )
            col = b * T + n * 512
            dst, dcol = {
                "q": (qT, col), "k": (kTt, col), "v": (vts[b], n * 512),
            }[which]
            nc.vector.tensor_copy(dst[:, dcol:dcol + 512], acc[:])
        # 'f1' kind: the PSUM accumulator stays open until f2 runs, so the
        # pop loop must emit f2 before any other ps_m user.
        return [(f1, 852, "f1"), (f2, 852, "f2")]

    def trans(b, s):
        """PE-transpose one 128-token strip of v into the augmented layout."""
        def f():
            tp = ps_m.tile([P, P], BF16, name="tr_ps", tag="m")
            nc.tensor.transpose(tp[:], vts[b][:, s * P:(s + 1) * P], ident[:])
            nc.vector.tensor_copy(vas[(b, "A")][:, s * 65:s * 65 + 64],
                                  tp[:, 0:64])
            nc.vector.tensor_copy(vas[(b, "B")][:, s * 65:s * 65 + 64],
                                  tp[:, 64:128])
        return [(f, 150, "x")]

    def qkv_chunks(b, parts=("q", "k", "v"), ns=range(NQ), with_trans=True):
        chunks = []
        if "k" in parts:
            for n in ns:
                chunks.extend(chain_parts(b, n, "k"))
        if "v" in parts:
            for n in ns:
                chunks.extend(chain_parts(b, n, "v"))
                if with_trans:
                    for s in range(4 * n, 4 * n + 4):
                        chunks.extend(trans(b, s))
        if "q" in parts:
            for n in ns:
                chunks.extend(chain_parts(b, n, "q"))
        return chunks

    # ---- per-q-tile state for drain/norm/outproj ----
    tile_state = {}

    def drain_norm(b, n, queue):
        """Right after the PV stop, before the av PSUM slots are re-claimed:
        fast reciprocals of the denominator rows (VectorE, direct from PSUM),
        DMA the unnormalized values out, broadcast the reciprocal rows via
        SBUF-to-SBUF DMA, one multiply to normalize. No PE instructions, so
        nothing here can stall the strip pipeline."""
        st = tile_state[(b, n)]
        avA, avB = st["avA"], st["avB"]
        # d-rows on VectorE, bulk copies on ScalarE: the av PSUM banks drain
        # in ~1.5us of parallel engine time, and the VectorE burst stays
        # small so QKV-chain casts queued behind it aren't delayed.
        dstg = dspool.tile([P, 1024], F32, name="dstg", tag="ds")
        nc.vector.tensor_copy(dstg[0:1, 0:512], avA[64:65, :])
        nc.vector.tensor_copy(dstg[0:1, 512:1024], avB[64:65, :])
        av_un = aupool.tile([P, 512], F32, name="av_un", tag="aun")
        nc.scalar.copy(av_un[0:64, :], avA[0:64, :])
        nc.scalar.copy(av_un[64:128, :], avB[0:64, :])
        # custom-DVE op only works at partition base 0
        rr = rrpool2.tile([P, 1024], F32, name="rr", tag="rr")
        nc.vector.reciprocal_approx_fast(out=rr[0:1, :], in_=dstg[0:1, :])
        rscr = drpool.tile([1, 1024], F32, name="rscr", tag="rscr")
        nc.sync.dma_start(out=rscr[:], in_=rr[0:1, :])
        bc = bcpool.tile([P, 512], F32, name="bc_sb", tag="bc")
        nc.sync.dma_start(out=bc[0:64, :],
                          in_=rscr[0:1, 0:512].to_broadcast((64, 512)))
        nc.gpsimd.dma_start(out=bc[64:128, :],
                            in_=rscr[0:1, 512:1024].to_broadcast((64, 512)))
        av_n = anpool.tile([P, 512], F32R, name="av_n", tag="an")
        nc.gpsimd.tensor_mul(av_n[:], av_un[:], bc[:])
        st["av_n"] = av_n
        for sub in range(4):
            queue.extend(outproj(b, n, sub))

    def outproj(b, n, sub):
        def f():
            av_n = tile_state[(b, n)]["av_n"]
            ot = otpool.tile([P, D], BF16, name="ot", tag="ot")
            for jh in range(2):
                op = ps_m.tile([P, 512], F32, name="op_ps", tag="m")
                nc.tensor.matmul(
                    op[:], av_n[:, sub * P:(sub + 1) * P],
                    wo_sb[:, jh * 512:(jh + 1) * 512],
                    start=True, stop=True,
                )
                nc.vector.tensor_copy(ot[:, jh * 512:(jh + 1) * 512], op[:])
            row0 = b * T + n * 512 + sub * P
            eng = nc.sync if sub % 2 == 0 else nc.gpsimd
            eng.dma_start(out=out[row0:row0 + P, :], in_=ot[:])
        return [(f, 500, "x")]

    # ---- exp-paced attention strip loop with credit-based filler ----
    CREDIT_PER_STRIP = 310
    CREDIT_CAP = 1000
    sched = {"credit": 0, "forced": None, "pv": None}

    def pump(queue, add):
        credit = min(sched["credit"] + add, CREDIT_CAP)
        if sched["forced"] is not None:
            f, cost, _ = sched["forced"]
            f()
            credit -= cost
            sched["forced"] = None
        while queue and queue[0][1] <= credit:
            f, cost, kind = queue.popleft()
            f()
            credit -= cost
            if kind == "f1":
                # matching f2 must be the next ps_m user
                sched["forced"] = queue.popleft()
                break
        sched["credit"] = credit

    def attn_batch(b, queue, on_tile_start=None):
        for n in range(NQ):
            if on_tile_start is not None:
                on_tile_start(n, queue)
            qcol = b * T + n * 512
            avA = ps_av.tile([P, 512], F32, name="avA", tag="av")
            avB = ps_av.tile([P, 512], F32, name="avB", tag="av")
            tile_state[(b, n)] = {"avA": avA, "avB": avB}
            for s in range(NS):
                kcol = b * T + s * P
                sc = ps_sc.tile([P, 1024], F32, name="sc", tag="sc")
                nc.tensor.matmul(
                    sc[:, 0:512], kTt[0:64, kcol:kcol + P],
                    qT[0:64, qcol:qcol + 512], start=True, stop=True,
                )
                nc.tensor.matmul(
                    sc[:, 512:1024], kTt[64:128, kcol:kcol + P],
                    qT[64:128, qcol:qcol + 512], start=True, stop=True,
                )
                ex = expool.tile([P, 1024], BF16, name="ex", tag="ex")
                nc.scalar.activation(out=ex[:], in_=sc[:], func=EXP,
                                     scale=0.125)
                # PE filler, then the PREVIOUS strip's PV matmuls: by the
                # time the PE reaches them, exp(s-1) is long done, so the
                # PE never waits on ScalarE and stays at max p-state.
                pump(queue, CREDIT_PER_STRIP)
                if sched["pv"] is not None:
                    sched["pv"]()

                def pv(s=s, ex=ex, avA=avA, avB=avB, b=b):
                    nc.tensor.matmul(
                        avA[0:65, :], vas[(b, "A")][:, s * 65:(s + 1) * 65],
                        ex[:, 0:512], start=(s == 0), stop=(s == NS - 1),
                    )
                    nc.tensor.matmul(
                        avB[0:65, :], vas[(b, "B")][:, s * 65:(s + 1) * 65],
                        ex[:, 512:1024], start=(s == 0), stop=(s == NS - 1),
                    )
                sched["pv"] = pv
            # boundary: one extra filler beat so the final PV's exp is done
            pump(queue, 500)
            sched["pv"]()
            sched["pv"] = None
            drain_norm(b, n, queue)
        return queue

    # ---- schedule ----
    # Phase 1: batch-0 QKV (PE-serial, overlapped with x DMA)
    for f, _c, _k in qkv_chunks(0):
        f()

    # Phase 2: attention batch 0; filler = batch-1 k/v chains + transposes
    # + q(b1, n0), then batch-0 outprojs as tiles complete.
    q2 = deque(qkv_chunks(1, parts=("k", "v")) +
               qkv_chunks(1, parts=("q",), ns=[0]))
    leftover = attn_batch(0, q2)

    # Phase 3: attention batch 1; filler = leftovers + q(b1, n+1) chains
    # (prepended at tile starts) + outprojs.
    def tile_start(n, queue):
        if n + 1 < NQ:
            for item in reversed(qkv_chunks(1, parts=("q",), ns=[n + 1])):
                queue.appendleft(item)

    q3 = leftover
    attn_batch(1, q3, on_tile_start=tile_start)

    # Tail: flush remaining filler (ends with the last tile's outproj).
    if sched["forced"] is not None:
        sched["forced"][0]()
        sched["forced"] = None
    while q3:
        f, _, kind = q3.popleft()
        f()


_PROGRAM = None


def _build_program():
    nc = bacc.Bacc(
        "TRN2", target_bir_lowering=False, debug=False,
        enable_asserts=False, num_devices=N_CORES,
    )
    xT = nc.dram_tensor("xT", [D, BT], BF16, kind="ExternalInput").ap()
    wq_in = nc.dram_tensor("wq_in", [P, D], BF16, kind="ExternalInput").ap()
    wk_in = nc.dram_tensor("wk_in", [P, D], BF16, kind="ExternalInput").ap()
    wv_in = nc.dram_tensor("wv_in", [P, D], BF16, kind="ExternalInput").ap()
    wo_in = nc.dram_tensor("wo_in", [P, D], F32R, kind="ExternalInput").ap()
    ident_in = nc.dram_tensor("ident_in", [P, P], BF16,
                              kind="ExternalInput").ap()
    ones_in = nc.dram_tensor("ones_in", [P, NS], BF16,
                             kind="ExternalInput").ap()
    out = nc.dram_tensor("out", [BT, D], BF16, kind="ExternalOutput").ap()
    with tile.TileContext(nc) as tc:
        _mhsa_kernel(tc, out, xT, wq_in, wk_in, wv_in, wo_in, ident_in,
                     ones_in)
    nc.compile()
    return nc


def get_program():
    global _PROGRAM
    if _PROGRAM is None:
        _PROGRAM = _build_program()
    return _PROGRAM


last_results = None


def _install_trace_hook():
    """Register the axon NTFF-profile hook that the agent image's antenv
    lacks, so run_bass_kernel_spmd(trace=True) can capture HW timings."""
    import sys
    import types

    if "antenv.axon_hooks" in sys.modules:
        return
    try:
        from trn_agent_boot.trn_boot import _ntff_profile_via_ctypes
        hook = _ntff_profile_via_ctypes("/opt/axon/libaxon_pjrt.so")
    except Exception:
        hook = None
    mod = types.ModuleType("antenv.axon_hooks")
    state = {"hook": hook}
    mod.get_axon_ntff_profile_hook = lambda: state["hook"]
    mod.set_axon_ntff_profile_hook = lambda h: state.__setitem__("hook", h)
    sys.modules["antenv.axon_hooks"] = mod

    import concourse.bass_utils as bu
    orig_upload = bu.upload_artifacts

    def safe_upload(tmpdir):
        try:
            return orig_upload(tmpdir)
        except Exception:
            return tmpdir

    bu.upload_artifacts = safe_upload


def kernel(x, W_q, W_k, W_v, W_o):
    global last_results
    import ml_dtypes
    bf16 = ml_dtypes.bfloat16

    x = np.ascontiguousarray(np.asarray(x, dtype=np.float32))
    W_q = np.asarray(W_q, dtype=np.float32)
    W_k = np.asarray(W_k, dtype=np.float32)
    W_v = np.asarray(W_v, dtype=np.float32)
    W_o = np.asarray(W_o, dtype=np.float32)

    xTn = np.ascontiguousarray(x.reshape(BT, D).T.astype(bf16))
    ident = np.eye(P, dtype=np.float32).astype(bf16)
    ones16 = np.ones((P, NS), dtype=np.float32).astype(bf16)

    def pack_w(w_rows):
        # [1024 in-dims, 128 out-dims] -> [128, 8*128] with the k-slice
        # index folded into the free dim (one contiguous DMA).
        wT = np.ascontiguousarray(w_rows.T)          # [1024, 128]
        return np.ascontiguousarray(
            wT.reshape(KT, P, P).transpose(1, 0, 2).reshape(P, D).astype(bf16)
        )

    in_maps = []
    for c in range(N_CORES):
        sl = slice(P * c, P * (c + 1))
        in_maps.append({
            "xT": xTn,
            "wq_in": pack_w(W_q[sl, :]),
            "wk_in": pack_w(W_k[sl, :]),
            "wv_in": pack_w(W_v[sl, :]),
            "wo_in": np.ascontiguousarray(W_o[:, sl].T),
            "ident_in": ident,
            "ones_in": ones16,
        })

    trace = bool(int(os.environ.get("KERNEL_TRACE", "0")))
    if trace:
        _install_trace_hook()
    nc = get_program()
    res = run_bass_kernel_spmd(
        nc, in_maps, core_ids=list(range(N_CORES)), trace=trace,
    )
    last_results = res
    total = res.results[0]["out"].astype(np.float32)
    for r in res.results[1:]:
        total = total + r["out"].astype(np.float32)
    return total.reshape(NB, T, D)
